# revision 1
# baseline (speedup 1.0000x reference)
"""Distributed cosine-attention kernel for TRN2 (8 NeuronCores).

Problem (nn_Attention): B=4, N=2048, D_MODEL=1024, HEADS=16, DIM_HEAD=64
  qkv = x @ w_qkv.T + b_qkv ; q,k l2-normalized over head dim;
  attn = softmax(clip-scale * qn @ kn^T); out = (attn @ v) @ w_out.T

Sharding: core c handles batch b=c//2 and global heads hg*8..hg*8+8 (hg=c%2).
Each core computes a partial out^T [D_MODEL, N]; the host sums the two cores
of each batch and transposes.

Per-core dataflow (no on-device transposes needed anywhere):
  - host passes x[b].T ("xT" [C,T]) and pre-transposed weight shards
  - QK proj -> Q^T/K^T [d-on-partition, tok-free], head pairs packed 64+64
  - V proj -> V [tok-on-partition, d-free]; bias via K=1 matmul
  - l2norm: sum of squares per head via mask matmul; rsqrt via Ln+Exp on
    ScalarE (single activation-table set); broadcast across partitions via
    step-0 DMA; logit scale folded into K^T
  - S^T tiles [keys, queries] via f32r row-packed matmuls (two K=64 heads
    in row groups 0/64); P^T = Exp(S^T - s) over 4-bank PSUM groups (bf16)
  - O^T = P@V and the softmax denominator via bf16 col-packed matmuls
    (tile_position (0,0)/(0,64)), PSUM-accumulated over all key tiles
  - out^T partial = woutT tiles @ O (bf16)
All emission is software-pipelined: chain ops for unit u-1 are emitted after
the bulk matmuls of unit u, so the in-order PE stream never stalls on
ACT/DVE round-trips.
"""
import sys
sys.path.insert(0, "/opt/trn_rl_repo")

from dataclasses import dataclass

import numpy as np

try:
    import ml_dtypes
    ml_bf16 = ml_dtypes.bfloat16
except ImportError:  # pragma: no cover
    ml_bf16 = np.float32

import concourse.bass as bass
import concourse.tile as tile
import concourse.mybir as mybir
from concourse import bacc
from concourse.bass_utils import run_bass_kernel_spmd

F32 = mybir.dt.float32
F32R = mybir.dt.float32r
BF16 = mybir.dt.bfloat16
AF = mybir.ActivationFunctionType

D_MODEL = 1024
HEADS = 16
DIM_HEAD = 64
INNER = HEADS * DIM_HEAD
B = 4
N = 2048
N_CORES = 8
LOG100 = float(np.log(100.0))

_ACT_SET = "natural_log_exp_and_others"
_tables_patched = False


def _patch_act_tables():
    """Make every activation resolve to one table set (it contains ln, exp,
    square, copy, identity) so no ACT_TABLE_LOAD thrash occurs."""
    global _tables_patched
    if _tables_patched:
        return
    orig = bacc.get_activation_tables

    def patched(arch):
        tabs = orig(arch)
        if _ACT_SET in tabs:
            tabs = {k: (v if k == _ACT_SET else set())
                    for k, v in tabs.items()}
        return tabs

    bacc.get_activation_tables = patched
    _tables_patched = True


@dataclass
class Cfg:
    T: int = N
    C: int = D_MODEL
    NH: int = 8
    DH: int = DIM_HEAD
    QB: int = 512
    SG: int = 2             # k-tiles per exp group
    merge_pairs: tuple = (True, True, True, True)

    @property
    def PAIRS(self):
        return self.NH // 2

    @property
    def CT(self):
        return self.C // 128

    @property
    def KT(self):
        return self.T // 128

    @property
    def NQB(self):
        return self.T // self.QB

    @property
    def VW(self):
        return self.NH * self.DH


def build(cfg: Cfg):
    _patch_act_tables()
    T, C, QB = cfg.T, cfg.C, cfg.QB
    PAIRS, CT, KT, NQB, VW = cfg.PAIRS, cfg.CT, cfg.KT, cfg.NQB, cfg.VW
    SG = cfg.SG

    nc = bacc.Bacc("TRN2", target_bir_lowering=False, debug=False,
                   enable_asserts=False)

    xT_d = nc.declare_dram_parameter("xT", [C, T], F32R, isOutput=False)
    wqkT_d = nc.declare_dram_parameter("wqkT", [C, 2 * PAIRS * 128], F32R,
                                       isOutput=False)
    bqk_d = nc.declare_dram_parameter("bqk", [2 * PAIRS, 128, 1], F32,
                                      isOutput=False)
    wvT_d = nc.declare_dram_parameter("wvT", [C, VW], F32R, isOutput=False)
    bv_d = nc.declare_dram_parameter("bv", [1, VW], F32R, isOutput=False)
    woT_d = nc.declare_dram_parameter("woT", [VW, C], BF16, isOutput=False)
    # per-head scale constants: [:, 0]=-s_h (exp bias), [:, 1]=ln(s_h)
    scl_d = nc.declare_dram_parameter("scl", [cfg.NH, 2, 1], F32,
                                      isOutput=False)
    sel2T_d = nc.declare_dram_parameter("sel2T", [128, 2], F32,
                                        isOutput=False)
    out_d = nc.declare_dram_parameter("out", [C, T], F32, isOutput=True)

    with tile.TileContext(nc) as tc:
        with (
            tc.tile_pool(name="const", bufs=1) as const,
            tc.tile_pool(name="dram", bufs=1, space="DRAM") as dram,
            tc.tile_pool(name="xt", bufs=1) as xt_pool,
            tc.tile_pool(name="wcol", bufs=2) as wcol_pool,
            tc.tile_pool(name="qksb", bufs=2) as qk_sb,
            tc.tile_pool(name="norm", bufs=2) as norm_sb,
            tc.tile_pool(name="vtmp", bufs=2) as vtmp_pool,
            tc.tile_pool(name="att", bufs=2) as att_sb,
            tc.tile_pool(name="pt", bufs=2) as pt_pool,
            tc.tile_pool(name="ofin", bufs=1) as ofin_pool,
            tc.tile_pool(name="otout", bufs=2) as ot_pool,
            # PSUM budget (8 banks): mm 2 + sg 4 + pv 1 + lb 1
            tc.tile_pool(name="psmm", bufs=2, space="PSUM") as ps_mm,
            tc.tile_pool(name="pssg", bufs=1, space="PSUM") as ps_sg,
            tc.tile_pool(name="pspv", bufs=1, space="PSUM") as ps_pv,
        ):
            # ---- DRAM spill tensors ----
            qhat_sp = [dram.tile([128, T], F32R, tag=f"qsp{p}",
                                 name=f"qsp{p}") for p in range(PAIRS)]
            khat_sp = [dram.tile([128, T], F32R, tag=f"ksp{p}",
                                 name=f"ksp{p}") for p in range(PAIRS)]
            vhat_sp = dram.tile([KT, 128, VW], BF16, tag="vsp")
            rq_sp = [dram.tile([2, QB], F32, tag=f"rqsp{i}", name=f"rqsp{i}")
                     for i in range(2)]
            rl_sp = [dram.tile([2, QB], F32, tag=f"rlsp{i}", name=f"rlsp{i}")
                     for i in range(2)]

            # ---- constants ----
            scratch_f = const.tile([128, 128], F32, tag="scratch")
            nc.vector.memset(scratch_f, 1.0)
            ones_r = const.tile([1, 128], F32R, tag="ones_r")
            nc.vector.tensor_copy(ones_r, scratch_f[0:1, :])
            ones_bf = const.tile([128, 64], BF16, tag="ones_bf")
            nc.vector.tensor_copy(ones_bf, scratch_f[:, 0:64])
            sel2T_f = const.tile([128, 2], F32, tag="sel2Tf")
            nc.sync.dma_start(out=sel2T_f, in_=sel2T_d.ap())
            sel2T = const.tile([128, 2], F32R, tag="sel2T")
            nc.vector.tensor_copy(sel2T, sel2T_f)

            nbias_cols = []
            for h in range(cfg.NH):
                col = const.tile([128, 1], F32, tag=f"nb{h}", name=f"nb{h}")
                nc.sync.dma_start(
                    out=col, in_=scl_d.ap()[h, 0:1, :].to_broadcast((128, 1)))
                nbias_cols.append(col)
            lns_cols = []
            for p in range(PAIRS):
                col = const.tile([2, 1], F32, tag=f"lns{p}", name=f"lns{p}")
                nc.sync.dma_start(out=col,
                                  in_=scl_d.ap()[2 * p:2 * p + 2, 1, :])
                lns_cols.append(col)
            zero_col = const.tile([2, 1], F32, tag="zeroc")
            nc.vector.memset(zero_col, 0.0)

            bqk_cols = []
            for it in range(2 * PAIRS):
                col = const.tile([128, 1], F32, tag=f"bqk{it}",
                                 name=f"bqk{it}")
                nc.sync.dma_start(out=col, in_=bqk_d.ap()[it])
                bqk_cols.append(col)
            bv_r = const.tile([1, VW], F32R, tag="bv")
            nc.sync.dma_start(out=bv_r, in_=bv_d.ap())

            wv_res = const.tile([128, CT, VW], F32R, tag="wv_res")
            nc.sync.dma_start(
                out=wv_res,
                in_=wvT_d.ap().rearrange("(ct p) v -> p ct v", p=128))
            wo_res = const.tile([128, PAIRS, C], BF16, tag="wo_res")
            nc.sync.dma_start(
                out=wo_res,
                in_=woT_d.ap().rearrange("(pt p) c -> p pt c", p=128))

            xt = []
            for ct in range(CT):
                t = xt_pool.tile([128, T], F32R, tag=f"xt{ct}",
                                 name=f"xt{ct}")
                nc.sync.dma_start(out=t,
                                  in_=xT_d.ap()[ct * 128:(ct + 1) * 128, :])
                xt.append(t)

            # ================= V projection (pipelined evac) ==============
            pend_v = None

            def flush_v():
                nonlocal pend_v
                if pend_v is None:
                    return
                tt, vps = pend_v
                vtmp = vtmp_pool.tile([128, VW], BF16, tag="vtmp",
                                      name=f"vtmp{tt}")
                nc.vector.tensor_copy(vtmp, vps)
                nc.sync.dma_start(out=vhat_sp[tt], in_=vtmp)
                pend_v = None

            for tt in range(KT):
                vps = ps_mm.tile([128, VW], F32, tag="mm", name=f"vps{tt}")
                for ct in range(CT):
                    nc.tensor.matmul(vps, xt[ct][:, tt * 128:(tt + 1) * 128],
                                     wv_res[:, ct, :], start=(ct == 0),
                                     stop=False)
                nc.tensor.matmul(vps, ones_r[:], bv_r[:], start=False,
                                 stop=True)
                flush_v()
                pend_v = (tt, vps)
            flush_v()

            # ============ QK projection + l2norm (pipelined) ============
            pend_qk = None

            def flush_qk():
                nonlocal pend_qk
                if pend_qk is None:
                    return
                p, is_k, tb, it, qs = pend_qk
                ts = slice(tb * QB, (tb + 1) * QB)
                uid = f"{it}_{tb}"
                qraw = qk_sb.tile([128, QB], F32, tag="qraw",
                                  name=f"qraw{uid}")
                nc.vector.tensor_scalar_add(qraw, qs, bqk_cols[it])
                q2 = qk_sb.tile([128, QB], F32R, tag="q2", name=f"q2{uid}")
                nc.vector.tensor_mul(q2, qraw, qraw)
                ss = ps_mm.tile([2, QB], F32, tag="mm", name=f"ss{uid}")
                nc.tensor.matmul(ss, sel2T[:], q2[:], start=True, stop=True)
                lss = norm_sb.tile([2, QB], F32, tag="lss", name=f"lss{uid}")
                nc.scalar.activation(lss, ss, AF.Ln)
                rq = norm_sb.tile([2, QB], F32, tag="rq", name=f"rq{uid}")
                nc.scalar.activation(rq, lss, AF.Exp, scale=-0.5,
                                     bias=lns_cols[p] if is_k
                                     else zero_col[:])
                rqd = rq_sp[(2 * tb + it) % 2]
                nc.sync.dma_start(out=rqd, in_=rq)
                rqbc = qk_sb.tile([128, QB], F32, tag="rqbc",
                                  name=f"rqbc{uid}")
                nc.sync.dma_start(out=rqbc[0:64, :],
                                  in_=rqd[0:1, :].to_broadcast((64, QB)))
                nc.sync.dma_start(out=rqbc[64:128, :],
                                  in_=rqd[1:2, :].to_broadcast((64, QB)))
                qhat = qk_sb.tile([128, QB], F32R, tag="qhat",
                                  name=f"qhat{uid}")
                nc.vector.tensor_mul(qhat, qraw, rqbc)
                dst = khat_sp[p] if is_k else qhat_sp[p]
                nc.sync.dma_start(out=dst[:, ts], in_=qhat)
                pend_qk = None

            def emit_proj_pair(p):
                nonlocal pend_qk
                for is_k in (0, 1):
                    it = 2 * p + is_k
                    wcol = wcol_pool.tile([128, CT, 128], F32R, tag="wcol",
                                          name=f"wcol{it}")
                    nc.sync.dma_start(
                        out=wcol,
                        in_=wqkT_d.ap().rearrange(
                            "(ct pp) i -> pp ct i", pp=128)[
                                :, :, it * 128:(it + 1) * 128])
                    for tb in range(NQB):
                        ts = slice(tb * QB, (tb + 1) * QB)
                        qs = ps_mm.tile([128, QB], F32, tag="mm",
                                        name=f"qs{it}_{tb}")
                        for ct in range(CT):
                            nc.tensor.matmul(qs, wcol[:, ct, :], xt[ct][:, ts],
                                             start=(ct == 0),
                                             stop=(ct == CT - 1))
                        flush_qk()
                        pend_qk = (p, is_k, tb, it, qs)

            # ================= attention (pipelined) =================
            o_fin = {}
            NSG = KT // SG

            def emit_att_pair(p):
                kk = att_sb.tile([128, T], F32R, tag="kk", name=f"kk{p}")
                nc.sync.dma_start(out=kk, in_=khat_sp[p])
                qq = att_sb.tile([128, T], F32R, tag="qq", name=f"qq{p}")
                nc.sync.dma_start(out=qq, in_=qhat_sp[p])
                vv = att_sb.tile([128, KT, 128], BF16, tag="vv",
                                 name=f"vv{p}")
                nc.sync.dma_start(
                    out=vv,
                    in_=vhat_sp[:, :, p * 128:(p + 1) * 128].rearrange(
                        "kt pp w -> pp kt w"))
                for qb in range(NQB):
                    qsl = slice(qb * QB, (qb + 1) * QB)
                    pv = ps_pv.tile([128, QB], F32, tag="pv",
                                    name=f"pv{p}_{qb}")
                    lb = ps_pv.tile([128, QB], F32, tag="lb",
                                    name=f"lb{p}_{qb}")

                    def emit_pvlb(g, ptile, pv=pv, lb=lb, vv=vv):
                        for j in range(SG):
                            kt = g * SG + j
                            first = kt == 0
                            last = kt == KT - 1
                            nc.tensor.matmul(pv[0:64, :], vv[:, kt, 0:64],
                                             ptile[:, 0, j, :], start=first,
                                             stop=last, tile_position=(0, 0))
                            nc.tensor.matmul(pv[64:128, :], vv[:, kt, 64:128],
                                             ptile[:, 1, j, :], start=first,
                                             stop=last, tile_position=(0, 64),
                                             skip_group_check=True)
                            nc.tensor.matmul(lb[0:64, :], ones_bf[:],
                                             ptile[:, 0, j, :], start=first,
                                             stop=last, tile_position=(0, 0))
                            nc.tensor.matmul(lb[64:128, :], ones_bf[:],
                                             ptile[:, 1, j, :], start=first,
                                             stop=last, tile_position=(0, 64),
                                             skip_group_check=True)

                    pend_att = None
                    for g in range(NSG):
                        sg = ps_sg.tile([128, 2, SG, QB], F32, tag="sg",
                                        name=f"sg{p}_{qb}_{g}")
                        for j in range(SG):
                            kt = g * SG + j
                            ksl = slice(kt * 128, (kt + 1) * 128)
                            nc.tensor.matmul(sg[:, 0, j, :], kk[0:64, ksl],
                                             qq[0:64, qsl], start=True,
                                             stop=True)
                            nc.tensor.matmul(sg[:, 1, j, :], kk[64:128, ksl],
                                             qq[64:128, qsl], start=True,
                                             stop=True)
                        ptile = pt_pool.tile([128, 2, SG, QB], BF16, tag="pt",
                                             name=f"pt{p}_{qb}_{g}")
                        if cfg.merge_pairs[p]:
                            nc.scalar.activation(ptile, sg, AF.Exp,
                                                 bias=nbias_cols[2 * p][:])
                        else:
                            nc.scalar.activation(ptile[:, 0], sg[:, 0],
                                                 AF.Exp,
                                                 bias=nbias_cols[2 * p][:])
                            nc.scalar.activation(
                                ptile[:, 1], sg[:, 1], AF.Exp,
                                bias=nbias_cols[2 * p + 1][:])
                        if pend_att is not None:
                            emit_pvlb(*pend_att)
                        pend_att = (g, ptile)
                    emit_pvlb(*pend_att)

                    rl = att_sb.tile([128, QB], F32, tag="rl",
                                     name=f"rl{p}_{qb}")
                    nc.vector.reciprocal_approx_fast(out=rl, in_=lb)
                    of = ofin_pool.tile([128, QB], BF16, tag=f"of{p}_{qb}",
                                        name=f"of{p}_{qb}")
                    nc.vector.tensor_mul(of, pv, rl)
                    o_fin[(p, qb)] = of

            # ================= out projection (pipelined evac) ============
            pend_o = None

            def flush_o():
                nonlocal pend_o
                if pend_o is None:
                    return
                qb, cb, op = pend_o
                csl = slice(cb * 128, (cb + 1) * 128)
                ot = ot_pool.tile([128, QB], F32, tag="ot",
                                  name=f"ot{qb}_{cb}")
                nc.vector.tensor_copy(ot, op)
                nc.sync.dma_start(
                    out=out_d.ap()[csl, qb * QB:(qb + 1) * QB], in_=ot)
                pend_o = None

            def emit_outproj_qb(qb):
                nonlocal pend_o
                for cb in range(CT):
                    op = ps_mm.tile([128, QB], F32, tag="mm",
                                    name=f"op{qb}_{cb}")
                    for p in range(PAIRS):
                        nc.tensor.matmul(op, wo_res[:, p,
                                                    cb * 128:(cb + 1) * 128],
                                         o_fin[(p, qb)][:],
                                         start=(p == 0), stop=(p == PAIRS - 1))
                    flush_o()
                    pend_o = (qb, cb, op)

            # ======== interleaved pair-level schedule ========
            for p in range(PAIRS):
                emit_proj_pair(p)
            flush_qk()
            for p in range(PAIRS):
                emit_att_pair(p)
            for qb in range(NQB):
                emit_outproj_qb(qb)
            flush_o()

    nc.compile()
    return nc


# ======================= host-side sharding =======================

def shard_inputs(x, w_qkv, b_qkv, w_out, logit_scale):
    x = np.ascontiguousarray(np.asarray(x, dtype=np.float32))
    w_qkv = np.asarray(w_qkv, dtype=np.float32)
    b_qkv = np.asarray(b_qkv, dtype=np.float32)
    w_out = np.asarray(w_out, dtype=np.float32)
    ls = np.asarray(logit_scale, dtype=np.float32).reshape(-1)
    s_all = np.exp(np.minimum(ls, LOG100)).astype(np.float32)

    Wq = w_qkv[0 * INNER:1 * INNER]
    Wk = w_qkv[1 * INNER:2 * INNER]
    Wv = w_qkv[2 * INNER:3 * INNER]
    bq = b_qkv[0 * INNER:1 * INNER]
    bk = b_qkv[1 * INNER:2 * INNER]
    bv = b_qkv[2 * INNER:3 * INNER]

    xT = [np.ascontiguousarray(x[b].T) for b in range(B)]

    per_hg = {}
    merge = [True] * 4
    for hg in range(2):
        heads = list(range(hg * 8, hg * 8 + 8))
        rows, brows = [], []
        for p in range(4):
            g0, g1 = heads[2 * p], heads[2 * p + 1]
            if s_all[g0] != s_all[g1]:
                merge[p] = False
            rows += [Wq[g0 * 64:(g0 + 1) * 64], Wq[g1 * 64:(g1 + 1) * 64],
                     Wk[g0 * 64:(g0 + 1) * 64], Wk[g1 * 64:(g1 + 1) * 64]]
            brows += [bq[g0 * 64:(g0 + 1) * 64], bq[g1 * 64:(g1 + 1) * 64],
                      bk[g0 * 64:(g0 + 1) * 64], bk[g1 * 64:(g1 + 1) * 64]]
        wqkT = np.ascontiguousarray(np.concatenate(rows, axis=0).T)
        bqk = np.ascontiguousarray(
            np.concatenate(brows, axis=0)).reshape(8, 128, 1)
        vsl = slice(hg * 512, (hg + 1) * 512)
        wvT = np.ascontiguousarray(Wv[vsl].T)
        bvs = np.ascontiguousarray(bv[vsl].reshape(1, 512))
        woT = np.ascontiguousarray(w_out[:, vsl].T.astype(ml_bf16))
        scl = np.stack([-s_all[heads], np.log(s_all[heads])],
                       axis=1).astype(np.float32).reshape(8, 2, 1)
        per_hg[hg] = dict(wqkT=wqkT, bqk=bqk, wvT=wvT, bv=bvs, woT=woT,
                          scl=scl)

    sel2 = np.zeros((2, 128), dtype=np.float32)
    sel2[0, 0:64] = 1.0
    sel2[1, 64:128] = 1.0
    sel2T = np.ascontiguousarray(sel2.T)
    in_maps = []
    for c in range(N_CORES):
        b, hg = c // 2, c % 2
        m = dict(per_hg[hg])
        m["xT"] = xT[b]
        m["sel2T"] = sel2T
        in_maps.append(m)
    return in_maps, tuple(merge)


_NC_CACHE = {}
TRACE = False
LAST_RESULT = None


def kernel(x, w_qkv, b_qkv, w_out, logit_scale):
    global LAST_RESULT
    in_maps, merge_pairs = shard_inputs(x, w_qkv, b_qkv, w_out, logit_scale)
    cfg = Cfg(merge_pairs=merge_pairs)
    if merge_pairs not in _NC_CACHE:
        _NC_CACHE[merge_pairs] = build(cfg)
    nc = _NC_CACHE[merge_pairs]
    res = run_bass_kernel_spmd(nc, in_maps, core_ids=list(range(N_CORES)),
                               trace=TRACE)
    LAST_RESULT = res
    outs = [res.results[c]["out"] for c in range(N_CORES)]
    full = np.empty((B, N, D_MODEL), dtype=np.float32)
    for b in range(B):
        full[b] = (outs[2 * b] + outs[2 * b + 1]).T
    return full



# revision 11
# speedup vs baseline: 1.4875x; 1.4875x over previous
"""Distributed cosine-attention kernel for TRN2 (8 NeuronCores), v2.

Problem (nn_Attention): B=4, N=2048, D_MODEL=1024, HEADS=16, DIM_HEAD=64
  qkv = x @ w_qkv.T + b_qkv ; q,k l2-normalized over head dim;
  attn = softmax(clip-scale * qn @ kn^T); out = (attn @ v) @ w_out.T

Sharding: core c handles batch b=c//2 and global heads hg*8..hg*8+8 (hg=c%2).
Each core computes a partial out^T [D_MODEL, N]; the host sums the two cores
of each batch and transposes.

v2 design (vs v1 baseline at 636us):
  - all matmuls bf16 (FWL weight loads); f32 PSUM accumulate
  - everything SBUF-resident: no DRAM spills for qhat/khat/vhat
  - score PSUM double-buffered ([128,2,512] x2) -> exp per kt, no WAR stall
  - l2norm rsqrt batched per pair: one Ln + one Exp on [4,4,512] (ACT),
    ss tiles staged via DVE copies
  - emission interleaving: QK proj of pair p+1 threaded through attention of
    pair p, out-proj threaded through attention of pair 3, so the PE never
    idles while the ACT exp stream (the ~300us floor) drains
  - V-proj bias via DVE tensor-tensor add on evac (no K=1 bias matmuls)
PSUM budget (8 banks): mm 2 + sg 2x2 + pv 1 + lb 1.
"""
import sys
sys.path.insert(0, "/opt/trn_rl_repo")

from dataclasses import dataclass

import numpy as np

try:
    import ml_dtypes
    ml_bf16 = ml_dtypes.bfloat16
except ImportError:  # pragma: no cover
    ml_bf16 = np.float32

import concourse.bass as bass
import concourse.tile as tile
import concourse.mybir as mybir
from concourse import bacc
from concourse.bass_utils import run_bass_kernel_spmd

F32 = mybir.dt.float32
BF16 = mybir.dt.bfloat16
AF = mybir.ActivationFunctionType

D_MODEL = 1024
HEADS = 16
DIM_HEAD = 64
INNER = HEADS * DIM_HEAD
B = 4
N = 2048
N_CORES = 8
LOG100 = float(np.log(100.0))

_ACT_SET = "natural_log_exp_and_others"
_tables_patched = False


def _patch_act_tables():
    """Make every activation resolve to one table set (it contains ln, exp,
    square, copy, identity) so no ACT_TABLE_LOAD thrash occurs."""
    global _tables_patched
    if _tables_patched:
        return
    orig = bacc.get_activation_tables

    def patched(arch):
        tabs = orig(arch)
        if _ACT_SET in tabs:
            tabs = {k: (v if k == _ACT_SET else set())
                    for k, v in tabs.items()}
        return tabs

    bacc.get_activation_tables = patched
    _tables_patched = True


@dataclass
class Cfg:
    T: int = N
    C: int = D_MODEL
    NH: int = 8
    DH: int = DIM_HEAD
    QB: int = 512
    merge_pairs: tuple = (True, True, True, True)

    @property
    def PAIRS(self):
        return self.NH // 2

    @property
    def CT(self):
        return self.C // 128

    @property
    def KT(self):
        return self.T // 128

    @property
    def NQB(self):
        return self.T // self.QB

    @property
    def VW(self):
        return self.NH * self.DH


def build(cfg: Cfg):
    _patch_act_tables()
    T, C, QB = cfg.T, cfg.C, cfg.QB
    PAIRS, CT, KT, NQB, VW = cfg.PAIRS, cfg.CT, cfg.KT, cfg.NQB, cfg.VW

    nc = bacc.Bacc("TRN2", target_bir_lowering=False, debug=False,
                   enable_asserts=False)

    xT_d = nc.declare_dram_parameter("xT", [C, T], BF16, isOutput=False)
    wqkT_d = nc.declare_dram_parameter("wqkT", [C, 2 * PAIRS * 128], BF16,
                                       isOutput=False)
    bqk_d = nc.declare_dram_parameter("bqk", [2 * PAIRS, 128, 1], F32,
                                      isOutput=False)
    wvT_d = nc.declare_dram_parameter("wvT", [C, VW], BF16, isOutput=False)
    bv_d = nc.declare_dram_parameter("bv", [1, VW], F32, isOutput=False)
    woT_d = nc.declare_dram_parameter("woT", [VW, C], BF16, isOutput=False)
    # per-head scale constants: [:, 0]=-s_h (exp bias), [:, 1]=ln(s_h)
    scl_d = nc.declare_dram_parameter("scl", [cfg.NH, 2, 1], F32,
                                      isOutput=False)
    # per-pair norm-exp bias col: rows 0,1 (q halves) = 0; rows 32,33
    # (k halves) = ln(s_h) so the k-side normalizer folds in the logit scale
    lnsb_d = nc.declare_dram_parameter("lnsb", [PAIRS, 34, 1], F32,
                                       isOutput=False)
    sel2T_d = nc.declare_dram_parameter("sel2T", [128, 2], BF16,
                                        isOutput=False)
    out_d = nc.declare_dram_parameter("out", [C, T], F32, isOutput=True)

    with tile.TileContext(nc) as tc:
        with (
            tc.tile_pool(name="const", bufs=1) as const,
            tc.tile_pool(name="dram", bufs=1, space="DRAM") as dram,
            tc.tile_pool(name="xt", bufs=1) as xt_pool,
            tc.tile_pool(name="persist", bufs=1) as persist,
            tc.tile_pool(name="wcol", bufs=2) as wcol_pool,
            tc.tile_pool(name="qkw", bufs=8) as qkw_pool,
            tc.tile_pool(name="norm", bufs=2) as norm_sb,
            tc.tile_pool(name="att", bufs=2) as att_sb,
            tc.tile_pool(name="pt", bufs=4) as pt_pool,
            tc.tile_pool(name="ofin", bufs=1) as ofin_pool,
            tc.tile_pool(name="otout", bufs=2) as ot_pool,
            # PSUM budget (8 banks): mm 2 + sg 2x2 + pv 1 + lb 1
            tc.tile_pool(name="psmm", bufs=2, space="PSUM") as ps_mm,
            tc.tile_pool(name="pssg", bufs=2, space="PSUM") as ps_sg,
            tc.tile_pool(name="pspv", bufs=1, space="PSUM") as ps_pv,
        ):
            # ---- DRAM spill tensors (norm factors only, for broadcast) ----
            rq_sp = [dram.tile([34, NQB, QB], F32, tag=f"rqsp{p}",
                               name=f"rqsp{p}") for p in range(PAIRS)]

            # ---- constants ----
            scratch_f = const.tile([128, 64], F32, tag="scratch")
            nc.vector.memset(scratch_f, 1.0)
            ones_bf = const.tile([128, 64], BF16, tag="ones_bf")
            nc.vector.tensor_copy(ones_bf, scratch_f)
            sel2T = const.tile([128, 2], BF16, tag="sel2T")
            nc.sync.dma_start(out=sel2T, in_=sel2T_d.ap())

            nbias_cols = []
            for h in range(cfg.NH):
                col = const.tile([128, 1], F32, tag=f"nb{h}", name=f"nb{h}")
                nc.sync.dma_start(
                    out=col, in_=scl_d.ap()[h, 0:1, :].to_broadcast((128, 1)))
                nbias_cols.append(col)
            lnsb_cols = []
            for p in range(PAIRS):
                col = const.tile([34, 1], F32, tag=f"lnsb{p}",
                                 name=f"lnsb{p}")
                nc.sync.dma_start(out=col, in_=lnsb_d.ap()[p])
                lnsb_cols.append(col)

            bqk_cols = []
            for it in range(2 * PAIRS):
                col = const.tile([128, 1], F32, tag=f"bqk{it}",
                                 name=f"bqk{it}")
                nc.sync.dma_start(out=col, in_=bqk_d.ap()[it])
                bqk_cols.append(col)
            bv_bc = const.tile([128, VW], F32, tag="bv_bc")
            nc.sync.dma_start(out=bv_bc,
                              in_=bv_d.ap().to_broadcast((128, VW)))

            wv_res = const.tile([128, CT, VW], BF16, tag="wv_res")
            nc.sync.dma_start(
                out=wv_res,
                in_=wvT_d.ap().rearrange("(ct p) v -> p ct v", p=128))
            wo_res = const.tile([128, PAIRS, C], BF16, tag="wo_res")
            nc.sync.dma_start(
                out=wo_res,
                in_=woT_d.ap().rearrange("(pt p) c -> p pt c", p=128))

            xt = []
            for ct in range(CT):
                t = xt_pool.tile([128, T], BF16, tag=f"xt{ct}",
                                 name=f"xt{ct}")
                nc.sync.dma_start(out=t,
                                  in_=xT_d.ap()[ct * 128:(ct + 1) * 128, :])
                xt.append(t)

            # ---- persistent SBUF tensors ----
            qhat = [persist.tile([128, T], BF16, tag=f"qh{p}",
                                 name=f"qh{p}") for p in range(PAIRS)]
            khat = [persist.tile([128, T], BF16, tag=f"kh{p}",
                                 name=f"kh{p}") for p in range(PAIRS)]
            # V, token-partition: [128, KT, VW]; pair p uses cols p*128..
            vv = persist.tile([128, KT, VW], BF16, tag="vv")

            # ================= V projection (16 tt units) =================
            def gen_vproj():
                pend = None
                for tt in range(KT):
                    vps = ps_mm.tile([128, VW], F32, tag="mm",
                                     name=f"vps{tt}")
                    for ct in range(CT):
                        nc.tensor.matmul(vps,
                                         xt[ct][:, tt * 128:(tt + 1) * 128],
                                         wv_res[:, ct, :], start=(ct == 0),
                                         stop=(ct == CT - 1))
                        yield
                    if pend is not None:
                        po, pt_ = pend
                        nc.vector.tensor_add(vv[:, pt_, :], po, bv_bc)
                    pend = (vps, tt)
                    yield
                po, pt_ = pend
                nc.vector.tensor_add(vv[:, pt_, :], po, bv_bc)
                yield

            # ============ QK projection + l2norm (per pair) ============
            # norm staging rows: q halves at partitions 0,1; k halves at
            # 32,33 (ss k-matmul col-tiled to position 32 so every engine
            # access keeps a 32-aligned partition offset)
            def gen_qk_pair(p):
                qraw_tiles = {}
                stage = norm_sb.tile([34, NQB, QB], F32, tag="stage",
                                     name=f"stage{p}", bufs=1)
                wcols = []
                for is_k in (0, 1):
                    it = 2 * p + is_k
                    wcol = wcol_pool.tile([128, CT, 128], BF16, tag="wcol",
                                          name=f"wcol{it}")
                    nc.sync.dma_start(
                        out=wcol,
                        in_=wqkT_d.ap().rearrange(
                            "(ct pp) i -> pp ct i", pp=128)[
                                :, :, it * 128:(it + 1) * 128])
                    wcols.append(wcol)
                yield

                def flush(is_k, tb, qs):
                    it = 2 * p + is_k
                    uid = f"{it}_{tb}"
                    qraw = qkw_pool.tile([128, QB], BF16, tag="qraw",
                                         name=f"qraw{uid}")
                    nc.vector.tensor_scalar_add(qraw, qs, bqk_cols[it])
                    q2 = qkw_pool.tile([128, QB], BF16, tag="q2",
                                       name=f"q2{uid}", bufs=2)
                    nc.vector.tensor_mul(q2, qraw, qraw)
                    qraw_tiles[(is_k, tb)] = qraw
                    return q2

                pend = None
                for tb in range(NQB):
                    ts = slice(tb * QB, (tb + 1) * QB)
                    q2s = []
                    for is_k in (0, 1):
                        qs = ps_mm.tile([128, QB], F32, tag="mm",
                                        name=f"qs{p}_{is_k}_{tb}")
                        for ct in range(CT):
                            nc.tensor.matmul(qs, wcols[is_k][:, ct, :],
                                             xt[ct][:, ts],
                                             start=(ct == 0),
                                             stop=(ct == CT - 1))
                            yield
                        q2s.append(flush(is_k, tb, qs))
                        yield
                        # ss matmuls of the *previous* tb (q2 ready on DVE
                        # well before the PE reaches these matmuls)
                        if pend is not None:
                            pq2s, ptb = pend
                            pend = None
                            ss = ps_mm.tile([34, QB], F32, tag="mm",
                                            name=f"ss{p}_{ptb}")
                            nc.tensor.matmul(ss[0:2, :], sel2T, pq2s[0],
                                             start=True, stop=True,
                                             tile_position=(0, 0))
                            nc.tensor.matmul(ss[32:34, :], sel2T, pq2s[1],
                                             start=True, stop=True,
                                             tile_position=(0, 32),
                                             skip_group_check=True)
                            nc.vector.tensor_copy(stage[:, ptb, :], ss)
                    pend = (q2s, tb)
                    yield
                pq2s, ptb = pend
                ss = ps_mm.tile([34, QB], F32, tag="mm", name=f"ss{p}_{ptb}")
                nc.tensor.matmul(ss[0:2, :], sel2T, pq2s[0], start=True,
                                 stop=True, tile_position=(0, 0))
                nc.tensor.matmul(ss[32:34, :], sel2T, pq2s[1], start=True,
                                 stop=True, tile_position=(0, 32),
                                 skip_group_check=True)
                nc.vector.tensor_copy(stage[:, ptb, :], ss)
                yield
                # batched rsqrt via Ln + Exp (stays in one ACT table set):
                # rq = exp(-0.5*ln(ss) + lnsb) = s_h * ss^-0.5 (k rows)
                lss = norm_sb.tile([34, NQB, QB], F32, tag="lss",
                                   name=f"lss{p}", bufs=1)
                nc.scalar.activation(lss, stage, AF.Ln)
                rq = norm_sb.tile([34, NQB, QB], F32, tag="rq",
                                  name=f"rq{p}", bufs=1)
                nc.scalar.activation(rq, lss, AF.Exp, scale=-0.5,
                                     bias=lnsb_cols[p])
                nc.sync.dma_start(out=rq_sp[p], in_=rq)
                yield
                for is_k in (0, 1):
                    for tb in range(NQB):
                        ts = slice(tb * QB, (tb + 1) * QB)
                        uid = f"{p}_{is_k}_{tb}"
                        r0 = 32 * is_k
                        rqbc = qkw_pool.tile([128, QB], F32, tag="rqbc",
                                             name=f"rqbc{uid}", bufs=2)
                        nc.sync.dma_start(
                            out=rqbc[0:64, :],
                            in_=rq_sp[p][r0:r0 + 1, tb, :]
                            .to_broadcast((64, QB)))
                        nc.sync.dma_start(
                            out=rqbc[64:128, :],
                            in_=rq_sp[p][r0 + 1:r0 + 2, tb, :]
                            .to_broadcast((64, QB)))
                        dst = khat[p] if is_k else qhat[p]
                        nc.vector.tensor_mul(dst[:, ts],
                                             qraw_tiles[(is_k, tb)], rqbc)
                        yield

            # ================= out projection (per qb) =================
            o_fin = {}

            def gen_outproj_qb(qb):
                pend = None
                for cb in range(CT):
                    op = ps_mm.tile([128, QB], F32, tag="mm",
                                    name=f"op{qb}_{cb}")
                    for p in range(PAIRS):
                        nc.tensor.matmul(op,
                                         wo_res[:, p,
                                                cb * 128:(cb + 1) * 128],
                                         o_fin[(p, qb)][:],
                                         start=(p == 0),
                                         stop=(p == PAIRS - 1))
                        yield
                    if pend is not None:
                        po, pcb = pend
                        ot = ot_pool.tile([128, QB], F32, tag="ot",
                                          name=f"ot{qb}_{pcb}")
                        nc.vector.tensor_copy(ot, po)
                        nc.sync.dma_start(
                            out=out_d.ap()[pcb * 128:(pcb + 1) * 128,
                                           qb * QB:(qb + 1) * QB], in_=ot)
                    pend = (op, cb)
                    yield
                po, pcb = pend
                ot = ot_pool.tile([128, QB], F32, tag="ot",
                                  name=f"ot{qb}_{pcb}")
                nc.vector.tensor_copy(ot, po)
                nc.sync.dma_start(
                    out=out_d.ap()[pcb * 128:(pcb + 1) * 128,
                                   qb * QB:(qb + 1) * QB], in_=ot)
                yield

            # ================= attention (per pair) =================
            class Filler:
                def __init__(self):
                    self.gens = []
                    self.done_count = 0

                def add(self, g):
                    self.gens.append(g)

                def pop(self, n):
                    while n > 0 and self.gens:
                        try:
                            next(self.gens[0])
                            n -= 1
                        except StopIteration:
                            self.gens.pop(0)

                def drain(self):
                    while self.gens:
                        try:
                            next(self.gens[0])
                        except StopIteration:
                            self.gens.pop(0)

            def emit_att_pair(p, fill, budget):
                """budget: filler thunks to interleave per kt group."""
                vsl = slice(p * 128, (p + 1) * 128)
                kk, qq = khat[p], qhat[p]
                n_groups = NQB * KT
                gi = 0
                emitted = 0.0
                for qb in range(NQB):
                    qsl = slice(qb * QB, (qb + 1) * QB)
                    pv = ps_pv.tile([128, QB], F32, tag="pv",
                                    name=f"pv{p}_{qb}")
                    lb = ps_pv.tile([128, QB], F32, tag="lb",
                                    name=f"lb{p}_{qb}")
                    pend = None

                    def emit_pvlb(kt, ptile, pv=pv, lb=lb):
                        first = kt == 0
                        last = kt == KT - 1
                        v0 = vv[:, kt, vsl][:, 0:64]
                        v1 = vv[:, kt, vsl][:, 64:128]
                        nc.tensor.matmul(pv[0:64, :], v0, ptile[:, 0, :],
                                         start=first, stop=last,
                                         tile_position=(0, 0))
                        nc.tensor.matmul(pv[64:128, :], v1, ptile[:, 1, :],
                                         start=first, stop=last,
                                         tile_position=(0, 64),
                                         skip_group_check=True)
                        nc.tensor.matmul(lb[0:64, :], ones_bf, ptile[:, 0, :],
                                         start=first, stop=last,
                                         tile_position=(0, 0))
                        nc.tensor.matmul(lb[64:128, :], ones_bf,
                                         ptile[:, 1, :],
                                         start=first, stop=last,
                                         tile_position=(0, 64),
                                         skip_group_check=True)

                    for kt in range(KT):
                        ksl = slice(kt * 128, (kt + 1) * 128)
                        sg = ps_sg.tile([128, 2, QB], F32, tag="sg",
                                        name=f"sg{p}_{qb}_{kt}")
                        nc.tensor.matmul(sg[:, 0, :], kk[0:64, ksl],
                                         qq[0:64, qsl], start=True,
                                         stop=True)
                        nc.tensor.matmul(sg[:, 1, :], kk[64:128, ksl],
                                         qq[64:128, qsl], start=True,
                                         stop=True)
                        ptile = pt_pool.tile([128, 2, QB], BF16, tag="pt",
                                             name=f"pt{p}_{qb}_{kt}")
                        if cfg.merge_pairs[p]:
                            nc.scalar.activation(ptile, sg, AF.Exp,
                                                 bias=nbias_cols[2 * p][:])
                        else:
                            nc.scalar.activation(ptile[:, 0], sg[:, 0],
                                                 AF.Exp,
                                                 bias=nbias_cols[2 * p][:])
                            nc.scalar.activation(
                                ptile[:, 1], sg[:, 1], AF.Exp,
                                bias=nbias_cols[2 * p + 1][:])
                        if pend is not None:
                            emit_pvlb(*pend)
                        pend = (kt, ptile)
                        gi += 1
                        want = budget * gi
                        k = int(want - emitted)
                        if k > 0:
                            fill.pop(k)
                            emitted += k
                    emit_pvlb(*pend)

                    rl = att_sb.tile([128, QB], F32, tag="rl",
                                     name=f"rl{p}_{qb}")
                    nc.vector.reciprocal_approx_fast(out=rl, in_=lb)
                    of = ofin_pool.tile([128, QB], BF16, tag=f"of{p}_{qb}",
                                        name=f"of{p}_{qb}")
                    nc.vector.tensor_mul(of, pv, rl)
                    o_fin[(p, qb)] = of
                    yield qb

            # ======== top-level schedule ========
            fill = Filler()
            # pair 0 QK proj + norm, emitted densely
            for _ in gen_qk_pair(0):
                pass
            # V projection, emitted densely
            for _ in gen_vproj():
                pass
            # attention p with QK proj of p+1 interleaved
            for p in range(PAIRS):
                if p + 1 < PAIRS:
                    fill.add(gen_qk_pair(p + 1))
                    budget = 1.5
                else:
                    budget = 3.2
                for done_qb in emit_att_pair(p, fill, budget):
                    if p == PAIRS - 1:
                        # all pairs done for this qb -> out proj becomes
                        # legal filler
                        fill.add(gen_outproj_qb(done_qb))
                fill.drain()

    nc.compile()
    return nc


# ======================= host-side sharding =======================

def shard_inputs(x, w_qkv, b_qkv, w_out, logit_scale):
    x = np.ascontiguousarray(np.asarray(x, dtype=np.float32))
    w_qkv = np.asarray(w_qkv, dtype=np.float32)
    b_qkv = np.asarray(b_qkv, dtype=np.float32)
    w_out = np.asarray(w_out, dtype=np.float32)
    ls = np.asarray(logit_scale, dtype=np.float32).reshape(-1)
    s_all = np.exp(np.minimum(ls, LOG100)).astype(np.float32)

    Wq = w_qkv[0 * INNER:1 * INNER]
    Wk = w_qkv[1 * INNER:2 * INNER]
    Wv = w_qkv[2 * INNER:3 * INNER]
    bq = b_qkv[0 * INNER:1 * INNER]
    bk = b_qkv[1 * INNER:2 * INNER]
    bv = b_qkv[2 * INNER:3 * INNER]

    xT = [np.ascontiguousarray(x[b].T.astype(ml_bf16)) for b in range(B)]

    per_hg = {}
    merge = [True] * 4
    for hg in range(2):
        heads = list(range(hg * 8, hg * 8 + 8))
        rows, brows = [], []
        lnsb = np.zeros((4, 34, 1), dtype=np.float32)
        for p in range(4):
            g0, g1 = heads[2 * p], heads[2 * p + 1]
            if s_all[g0] != s_all[g1]:
                merge[p] = False
            rows += [Wq[g0 * 64:(g0 + 1) * 64], Wq[g1 * 64:(g1 + 1) * 64],
                     Wk[g0 * 64:(g0 + 1) * 64], Wk[g1 * 64:(g1 + 1) * 64]]
            brows += [bq[g0 * 64:(g0 + 1) * 64], bq[g1 * 64:(g1 + 1) * 64],
                      bk[g0 * 64:(g0 + 1) * 64], bk[g1 * 64:(g1 + 1) * 64]]
            lnsb[p, 32, 0] = np.log(s_all[g0])
            lnsb[p, 33, 0] = np.log(s_all[g1])
        wqkT = np.ascontiguousarray(
            np.concatenate(rows, axis=0).T.astype(ml_bf16))
        bqk = np.ascontiguousarray(
            np.concatenate(brows, axis=0)).reshape(8, 128, 1)
        vsl = slice(hg * 512, (hg + 1) * 512)
        wvT = np.ascontiguousarray(Wv[vsl].T.astype(ml_bf16))
        bvs = np.ascontiguousarray(bv[vsl].reshape(1, 512))
        woT = np.ascontiguousarray(w_out[:, vsl].T.astype(ml_bf16))
        scl = np.stack([-s_all[heads], np.log(s_all[heads])],
                       axis=1).astype(np.float32).reshape(8, 2, 1)
        per_hg[hg] = dict(wqkT=wqkT, bqk=bqk, wvT=wvT, bv=bvs, woT=woT,
                          scl=scl, lnsb=lnsb)

    sel2 = np.zeros((2, 128), dtype=np.float32)
    sel2[0, 0:64] = 1.0
    sel2[1, 64:128] = 1.0
    sel2T = np.ascontiguousarray(sel2.T.astype(ml_bf16))
    in_maps = []
    for c in range(N_CORES):
        b, hg = c // 2, c % 2
        m = dict(per_hg[hg])
        m["xT"] = xT[b]
        m["sel2T"] = sel2T
        in_maps.append(m)
    return in_maps, tuple(merge)


_NC_CACHE = {}
TRACE = False
LAST_RESULT = None


def kernel(x, w_qkv, b_qkv, w_out, logit_scale):
    global LAST_RESULT
    in_maps, merge_pairs = shard_inputs(x, w_qkv, b_qkv, w_out, logit_scale)
    cfg = Cfg(merge_pairs=merge_pairs)
    if merge_pairs not in _NC_CACHE:
        _NC_CACHE[merge_pairs] = build(cfg)
    nc = _NC_CACHE[merge_pairs]
    res = run_bass_kernel_spmd(nc, in_maps, core_ids=list(range(N_CORES)),
                               trace=TRACE)
    LAST_RESULT = res
    outs = [res.results[c]["out"] for c in range(N_CORES)]
    full = np.empty((B, N, D_MODEL), dtype=np.float32)
    for b in range(B):
        full[b] = (outs[2 * b] + outs[2 * b + 1]).T
    return full


# revision 15
# speedup vs baseline: 1.5457x; 1.0391x over previous
"""Distributed cosine-attention kernel for TRN2 (8 NeuronCores), v2.

Problem (nn_Attention): B=4, N=2048, D_MODEL=1024, HEADS=16, DIM_HEAD=64
  qkv = x @ w_qkv.T + b_qkv ; q,k l2-normalized over head dim;
  attn = softmax(clip-scale * qn @ kn^T); out = (attn @ v) @ w_out.T

Sharding: core c handles batch b=c//2 and global heads hg*8..hg*8+8 (hg=c%2).
Each core computes a partial out^T [D_MODEL, N]; the host sums the two cores
of each batch and transposes.

v2 design (vs v1 baseline at 636us):
  - all matmuls bf16 (FWL weight loads); f32 PSUM accumulate
  - everything SBUF-resident: no DRAM spills for qhat/khat/vhat
  - score PSUM double-buffered ([128,2,512] x2) -> exp per kt, no WAR stall
  - l2norm rsqrt batched per pair: one Ln + one Exp on [4,4,512] (ACT),
    ss tiles staged via DVE copies
  - emission interleaving: QK proj of pair p+1 threaded through attention of
    pair p, out-proj threaded through attention of pair 3, so the PE never
    idles while the ACT exp stream (the ~300us floor) drains
  - V-proj bias via DVE tensor-tensor add on evac (no K=1 bias matmuls)
PSUM budget (8 banks): mm 2 + sg 2x2 + pv 1 + lb 1.
"""
import sys
sys.path.insert(0, "/opt/trn_rl_repo")

from dataclasses import dataclass

import numpy as np

try:
    import ml_dtypes
    ml_bf16 = ml_dtypes.bfloat16
except ImportError:  # pragma: no cover
    ml_bf16 = np.float32

import concourse.bass as bass
import concourse.tile as tile
import concourse.mybir as mybir
from concourse import bacc
from concourse.bass_utils import run_bass_kernel_spmd

F32 = mybir.dt.float32
BF16 = mybir.dt.bfloat16
AF = mybir.ActivationFunctionType

D_MODEL = 1024
HEADS = 16
DIM_HEAD = 64
INNER = HEADS * DIM_HEAD
B = 4
N = 2048
N_CORES = 8
LOG100 = float(np.log(100.0))

_ACT_SET = "natural_log_exp_and_others"
_tables_patched = False


def _patch_act_tables():
    """Make every activation resolve to one table set (it contains ln, exp,
    square, copy, identity) so no ACT_TABLE_LOAD thrash occurs."""
    global _tables_patched
    if _tables_patched:
        return
    orig = bacc.get_activation_tables

    def patched(arch):
        tabs = orig(arch)
        if _ACT_SET in tabs:
            tabs = {k: (v if k == _ACT_SET else set())
                    for k, v in tabs.items()}
        return tabs

    bacc.get_activation_tables = patched
    _tables_patched = True


@dataclass
class Cfg:
    T: int = N
    C: int = D_MODEL
    NH: int = 8
    DH: int = DIM_HEAD
    QB: int = 512
    merge_pairs: tuple = (True, True, True, True)

    @property
    def PAIRS(self):
        return self.NH // 2

    @property
    def CT(self):
        return self.C // 128

    @property
    def KT(self):
        return self.T // 128

    @property
    def NQB(self):
        return self.T // self.QB

    @property
    def VW(self):
        return self.NH * self.DH


def build(cfg: Cfg):
    _patch_act_tables()
    T, C, QB = cfg.T, cfg.C, cfg.QB
    PAIRS, CT, KT, NQB, VW = cfg.PAIRS, cfg.CT, cfg.KT, cfg.NQB, cfg.VW

    nc = bacc.Bacc("TRN2", target_bir_lowering=False, debug=False,
                   enable_asserts=False)

    xT_d = nc.declare_dram_parameter("xT", [C, T], BF16, isOutput=False)
    wqkT_d = nc.declare_dram_parameter("wqkT", [C, 2 * PAIRS * 128], BF16,
                                       isOutput=False)
    bqk_d = nc.declare_dram_parameter("bqk", [2 * PAIRS, 128, 1], F32,
                                      isOutput=False)
    wvT_d = nc.declare_dram_parameter("wvT", [C, VW], BF16, isOutput=False)
    bv_d = nc.declare_dram_parameter("bv", [1, VW], F32, isOutput=False)
    woT_d = nc.declare_dram_parameter("woT", [VW, C], BF16, isOutput=False)
    # per-head scale constants: [:, 0]=-s_h (exp bias), [:, 1]=ln(s_h)
    scl_d = nc.declare_dram_parameter("scl", [cfg.NH, 2, 1], F32,
                                      isOutput=False)
    # per-pair norm-exp bias col: rows 0,1 (q halves) = 0; rows 32,33
    # (k halves) = ln(s_h) so the k-side normalizer folds in the logit scale
    lnsb_d = nc.declare_dram_parameter("lnsb", [PAIRS, 34, 1], F32,
                                       isOutput=False)
    sel2T_d = nc.declare_dram_parameter("sel2T", [128, 2], BF16,
                                        isOutput=False)
    out_d = nc.declare_dram_parameter("out", [C, T], F32, isOutput=True)

    with tile.TileContext(nc) as tc:
        with (
            tc.tile_pool(name="const", bufs=1) as const,
            tc.tile_pool(name="dram", bufs=1, space="DRAM") as dram,
            tc.tile_pool(name="xt", bufs=1) as xt_pool,
            tc.tile_pool(name="persist", bufs=1) as persist,
            tc.tile_pool(name="wcol", bufs=2) as wcol_pool,
            tc.tile_pool(name="qkw", bufs=8) as qkw_pool,
            tc.tile_pool(name="norm", bufs=2) as norm_sb,
            tc.tile_pool(name="att", bufs=2) as att_sb,
            tc.tile_pool(name="pt", bufs=4) as pt_pool,
            tc.tile_pool(name="ofin", bufs=1) as ofin_pool,
            tc.tile_pool(name="otout", bufs=2) as ot_pool,
            # PSUM budget (8 banks): mm 2 + sg 2x2 + pv 1 + lb 1
            tc.tile_pool(name="psmm", bufs=2, space="PSUM") as ps_mm,
            tc.tile_pool(name="pssg", bufs=2, space="PSUM") as ps_sg,
            tc.tile_pool(name="pspv", bufs=1, space="PSUM") as ps_pv,
        ):
            # ---- DRAM spill tensors (norm factors only, for broadcast) ----
            rq_sp = [dram.tile([34, NQB, QB], F32, tag=f"rqsp{p}",
                               name=f"rqsp{p}") for p in range(PAIRS)]

            # ---- constants ----
            scratch_f = const.tile([128, 64], F32, tag="scratch")
            nc.vector.memset(scratch_f, 1.0)
            ones_bf = const.tile([128, 64], BF16, tag="ones_bf")
            nc.vector.tensor_copy(ones_bf, scratch_f)
            sel2T = const.tile([128, 2], BF16, tag="sel2T")
            nc.sync.dma_start(out=sel2T, in_=sel2T_d.ap())

            nbias_cols = []
            for h in range(cfg.NH):
                col = const.tile([128, 1], F32, tag=f"nb{h}", name=f"nb{h}")
                nc.sync.dma_start(
                    out=col, in_=scl_d.ap()[h, 0:1, :].to_broadcast((128, 1)))
                nbias_cols.append(col)
            lnsb_cols = []
            for p in range(PAIRS):
                col = const.tile([34, 1], F32, tag=f"lnsb{p}",
                                 name=f"lnsb{p}")
                nc.sync.dma_start(out=col, in_=lnsb_d.ap()[p])
                lnsb_cols.append(col)

            bqk_cols = []
            for it in range(2 * PAIRS):
                col = const.tile([128, 1], F32, tag=f"bqk{it}",
                                 name=f"bqk{it}")
                nc.sync.dma_start(out=col, in_=bqk_d.ap()[it])
                bqk_cols.append(col)
            bv_bc = const.tile([128, VW], F32, tag="bv_bc")
            nc.sync.dma_start(out=bv_bc,
                              in_=bv_d.ap().to_broadcast((128, VW)))

            # x tokens first, split in halves so all DMA queues carry x
            # during startup; weight tiles are deferred off the startup path
            xt = []
            for ct in range(CT):
                t = xt_pool.tile([128, T], BF16, tag=f"xt{ct}",
                                 name=f"xt{ct}")
                h = T // 2
                nc.sync.dma_start(
                    out=t[:, 0:h],
                    in_=xT_d.ap()[ct * 128:(ct + 1) * 128, 0:h])
                nc.sync.dma_start(
                    out=t[:, h:T],
                    in_=xT_d.ap()[ct * 128:(ct + 1) * 128, h:T])
                xt.append(t)

            wv_res = const.tile([128, CT, VW], BF16, tag="wv_res")
            wo_res = const.tile([128, PAIRS, C], BF16, tag="wo_res")

            def emit_wv_dma():
                nc.sync.dma_start(
                    out=wv_res,
                    in_=wvT_d.ap().rearrange("(ct p) v -> p ct v", p=128))

            def emit_wo_dma():
                nc.sync.dma_start(
                    out=wo_res,
                    in_=woT_d.ap().rearrange("(pt p) c -> p pt c", p=128))

            # ---- persistent SBUF tensors ----
            qhat = [persist.tile([128, T], BF16, tag=f"qh{p}",
                                 name=f"qh{p}") for p in range(PAIRS)]
            khat = [persist.tile([128, T], BF16, tag=f"kh{p}",
                                 name=f"kh{p}") for p in range(PAIRS)]
            # V, token-partition: [128, KT, VW]; pair p uses cols p*128..
            vv = persist.tile([128, KT, VW], BF16, tag="vv")

            # ================= V projection (16 tt units) =================
            def gen_vproj():
                pend = None
                for tt in range(KT):
                    vps = ps_mm.tile([128, VW], F32, tag="mm",
                                     name=f"vps{tt}")
                    for ct in range(CT):
                        nc.tensor.matmul(vps,
                                         xt[ct][:, tt * 128:(tt + 1) * 128],
                                         wv_res[:, ct, :], start=(ct == 0),
                                         stop=(ct == CT - 1))
                        yield
                    if pend is not None:
                        po, pt_ = pend
                        nc.vector.tensor_add(vv[:, pt_, :], po, bv_bc)
                    pend = (vps, tt)
                    yield
                po, pt_ = pend
                nc.vector.tensor_add(vv[:, pt_, :], po, bv_bc)
                yield

            # ============ QK projection + l2norm (per pair) ============
            # norm staging rows: q halves at partitions 0,1; k halves at
            # 32,33 (ss k-matmul col-tiled to position 32 so every engine
            # access keeps a 32-aligned partition offset)
            def gen_qk_pair(p, dense=False):
                qraw_tiles = {}
                q2_pairs = {tb: [None, None] for tb in range(NQB)}
                stage = norm_sb.tile([34, NQB, QB], F32, tag="stage",
                                     name=f"stage{p}", bufs=1)
                wcols = []
                for is_k in (0, 1):
                    it = 2 * p + is_k
                    wcol = wcol_pool.tile([128, CT, 128], BF16, tag="wcol",
                                          name=f"wcol{it}")
                    nc.sync.dma_start(
                        out=wcol,
                        in_=wqkT_d.ap().rearrange(
                            "(ct pp) i -> pp ct i", pp=128)[
                                :, :, it * 128:(it + 1) * 128])
                    wcols.append(wcol)
                yield

                def flush(is_k, tb, qs):
                    it = 2 * p + is_k
                    uid = f"{it}_{tb}"
                    qraw = qkw_pool.tile([128, QB], BF16, tag="qraw",
                                         name=f"qraw{uid}")
                    nc.vector.tensor_scalar_add(qraw, qs, bqk_cols[it])
                    q2 = qkw_pool.tile([128, QB], BF16, tag="q2",
                                       name=f"q2{uid}", bufs=8)
                    nc.vector.tensor_mul(q2, qraw, qraw)
                    qraw_tiles[(is_k, tb)] = qraw
                    q2_pairs[tb][is_k] = q2

                def emit_ss(tb):
                    ss = ps_mm.tile([34, QB], F32, tag="mm",
                                    name=f"ss{p}_{tb}")
                    nc.tensor.matmul(ss[0:2, :], sel2T, q2_pairs[tb][0],
                                     start=True, stop=True,
                                     tile_position=(0, 0))
                    nc.tensor.matmul(ss[32:34, :], sel2T, q2_pairs[tb][1],
                                     start=True, stop=True,
                                     tile_position=(0, 32),
                                     skip_group_check=True)
                    nc.vector.tensor_copy(stage[:, tb, :], ss)

                if dense:
                    # ct-major: PE starts as soon as xt[0] lands; 4 token
                    # blocks accumulate in the (idle) sg-tag PSUM banks
                    for is_k in (0, 1):
                        qsA = ps_sg.tile([128, 2, QB], F32, tag="sg",
                                         name=f"qsA{p}_{is_k}")
                        qsB = ps_sg.tile([128, 2, QB], F32, tag="sg",
                                         name=f"qsB{p}_{is_k}")
                        accs = [qsA[:, 0, :], qsA[:, 1, :],
                                qsB[:, 0, :], qsB[:, 1, :]]
                        for ct in range(CT):
                            for tb in range(NQB):
                                ts = slice(tb * QB, (tb + 1) * QB)
                                nc.tensor.matmul(accs[tb],
                                                 wcols[is_k][:, ct, :],
                                                 xt[ct][:, ts],
                                                 start=(ct == 0),
                                                 stop=(ct == CT - 1))
                                yield
                        for tb in range(NQB):
                            flush(is_k, tb, accs[tb])
                            yield
                    for tb in range(NQB):
                        emit_ss(tb)
                        yield
                else:
                    pend = None
                    for tb in range(NQB):
                        ts = slice(tb * QB, (tb + 1) * QB)
                        for is_k in (0, 1):
                            qs = ps_mm.tile([128, QB], F32, tag="mm",
                                            name=f"qs{p}_{is_k}_{tb}")
                            for ct in range(CT):
                                nc.tensor.matmul(qs, wcols[is_k][:, ct, :],
                                                 xt[ct][:, ts],
                                                 start=(ct == 0),
                                                 stop=(ct == CT - 1))
                                yield
                            flush(is_k, tb, qs)
                            yield
                            # ss matmuls of the *previous* tb (q2 ready on
                            # DVE well before the PE reaches these matmuls)
                            if pend is not None:
                                emit_ss(pend)
                                pend = None
                        pend = tb
                        yield
                    emit_ss(pend)
                    yield
                # batched rsqrt via Ln + Exp (stays in one ACT table set):
                # rq = exp(-0.5*ln(ss) + lnsb) = s_h * ss^-0.5 (k rows)
                lss = norm_sb.tile([34, NQB, QB], F32, tag="lss",
                                   name=f"lss{p}", bufs=1)
                nc.scalar.activation(lss, stage, AF.Ln)
                rq = norm_sb.tile([34, NQB, QB], F32, tag="rq",
                                  name=f"rq{p}", bufs=1)
                nc.scalar.activation(rq, lss, AF.Exp, scale=-0.5,
                                     bias=lnsb_cols[p])
                nc.sync.dma_start(out=rq_sp[p], in_=rq)
                yield
                for is_k in (0, 1):
                    for tb in range(NQB):
                        ts = slice(tb * QB, (tb + 1) * QB)
                        uid = f"{p}_{is_k}_{tb}"
                        r0 = 32 * is_k
                        rqbc = qkw_pool.tile([128, QB], F32, tag="rqbc",
                                             name=f"rqbc{uid}", bufs=2)
                        nc.sync.dma_start(
                            out=rqbc[0:64, :],
                            in_=rq_sp[p][r0:r0 + 1, tb, :]
                            .to_broadcast((64, QB)))
                        nc.sync.dma_start(
                            out=rqbc[64:128, :],
                            in_=rq_sp[p][r0 + 1:r0 + 2, tb, :]
                            .to_broadcast((64, QB)))
                        dst = khat[p] if is_k else qhat[p]
                        nc.vector.tensor_mul(dst[:, ts],
                                             qraw_tiles[(is_k, tb)], rqbc)
                        yield

            # ================= out projection (per qb) =================
            o_fin = {}

            def gen_outproj_qb(qb):
                pend = None
                for cb in range(CT):
                    op = ps_mm.tile([128, QB], F32, tag="mm",
                                    name=f"op{qb}_{cb}")
                    for p in range(PAIRS):
                        nc.tensor.matmul(op,
                                         wo_res[:, p,
                                                cb * 128:(cb + 1) * 128],
                                         o_fin[(p, qb)][:],
                                         start=(p == 0),
                                         stop=(p == PAIRS - 1))
                        yield
                    if pend is not None:
                        po, pcb = pend
                        ot = ot_pool.tile([128, QB], F32, tag="ot",
                                          name=f"ot{qb}_{pcb}")
                        nc.vector.tensor_copy(ot, po)
                        nc.sync.dma_start(
                            out=out_d.ap()[pcb * 128:(pcb + 1) * 128,
                                           qb * QB:(qb + 1) * QB], in_=ot)
                    pend = (op, cb)
                    yield
                po, pcb = pend
                ot = ot_pool.tile([128, QB], F32, tag="ot",
                                  name=f"ot{qb}_{pcb}")
                nc.vector.tensor_copy(ot, po)
                nc.sync.dma_start(
                    out=out_d.ap()[pcb * 128:(pcb + 1) * 128,
                                   qb * QB:(qb + 1) * QB], in_=ot)
                yield

            # ================= attention (per pair) =================
            class Filler:
                def __init__(self):
                    self.gens = []
                    self.done_count = 0

                def add(self, g):
                    self.gens.append(g)

                def pop(self, n):
                    while n > 0 and self.gens:
                        try:
                            next(self.gens[0])
                            n -= 1
                        except StopIteration:
                            self.gens.pop(0)

                def drain(self):
                    while self.gens:
                        try:
                            next(self.gens[0])
                        except StopIteration:
                            self.gens.pop(0)

            def emit_att_pair(p, fill, budget):
                """budget: filler thunks to interleave per kt group."""
                vsl = slice(p * 128, (p + 1) * 128)
                kk, qq = khat[p], qhat[p]
                n_groups = NQB * KT
                gi = 0
                emitted = 0.0
                for qb in range(NQB):
                    qsl = slice(qb * QB, (qb + 1) * QB)
                    pv = ps_pv.tile([128, QB], F32, tag="pv",
                                    name=f"pv{p}_{qb}")
                    lb = ps_pv.tile([128, QB], F32, tag="lb",
                                    name=f"lb{p}_{qb}")
                    pend = None

                    def emit_pvlb(kt, ptile, pv=pv, lb=lb):
                        first = kt == 0
                        last = kt == KT - 1
                        v0 = vv[:, kt, vsl][:, 0:64]
                        v1 = vv[:, kt, vsl][:, 64:128]
                        nc.tensor.matmul(pv[0:64, :], v0, ptile[:, 0, :],
                                         start=first, stop=last,
                                         tile_position=(0, 0))
                        nc.tensor.matmul(pv[64:128, :], v1, ptile[:, 1, :],
                                         start=first, stop=last,
                                         tile_position=(0, 64),
                                         skip_group_check=True)
                        nc.tensor.matmul(lb[0:64, :], ones_bf, ptile[:, 0, :],
                                         start=first, stop=last,
                                         tile_position=(0, 0))
                        nc.tensor.matmul(lb[64:128, :], ones_bf,
                                         ptile[:, 1, :],
                                         start=first, stop=last,
                                         tile_position=(0, 64),
                                         skip_group_check=True)

                    for kt in range(KT):
                        ksl = slice(kt * 128, (kt + 1) * 128)
                        sg = ps_sg.tile([128, 2, QB], F32, tag="sg",
                                        name=f"sg{p}_{qb}_{kt}")
                        nc.tensor.matmul(sg[:, 0, :], kk[0:64, ksl],
                                         qq[0:64, qsl], start=True,
                                         stop=True)
                        nc.tensor.matmul(sg[:, 1, :], kk[64:128, ksl],
                                         qq[64:128, qsl], start=True,
                                         stop=True)
                        ptile = pt_pool.tile([128, 2, QB], BF16, tag="pt",
                                             name=f"pt{p}_{qb}_{kt}")
                        if cfg.merge_pairs[p]:
                            nc.scalar.activation(ptile, sg, AF.Exp,
                                                 bias=nbias_cols[2 * p][:])
                        else:
                            nc.scalar.activation(ptile[:, 0], sg[:, 0],
                                                 AF.Exp,
                                                 bias=nbias_cols[2 * p][:])
                            nc.scalar.activation(
                                ptile[:, 1], sg[:, 1], AF.Exp,
                                bias=nbias_cols[2 * p + 1][:])
                        if pend is not None:
                            emit_pvlb(*pend)
                        pend = (kt, ptile)
                        gi += 1
                        want = budget * gi
                        k = int(want - emitted)
                        if k > 0:
                            fill.pop(k)
                            emitted += k
                    emit_pvlb(*pend)

                    rl = att_sb.tile([128, QB], F32, tag="rl",
                                     name=f"rl{p}_{qb}")
                    nc.vector.reciprocal_approx_fast(out=rl, in_=lb)
                    of = ofin_pool.tile([128, QB], BF16, tag=f"of{p}_{qb}",
                                        name=f"of{p}_{qb}")
                    nc.vector.tensor_mul(of, pv, rl)
                    o_fin[(p, qb)] = of
                    yield qb

            # ======== top-level schedule ========
            fill = Filler()
            # pair 0 QK proj + norm, emitted densely (ct-major)
            for _ in gen_qk_pair(0, dense=True):
                pass
            emit_wv_dma()
            # V projection, emitted densely
            for _ in gen_vproj():
                pass
            # attention p with QK proj of p+1 interleaved
            for p in range(PAIRS):
                if p == 2:
                    emit_wo_dma()
                if p + 1 < PAIRS:
                    fill.add(gen_qk_pair(p + 1))
                    budget = 2.0
                else:
                    budget = 3.2
                for done_qb in emit_att_pair(p, fill, budget):
                    if p == PAIRS - 1:
                        # all pairs done for this qb -> out proj becomes
                        # legal filler
                        fill.add(gen_outproj_qb(done_qb))
                fill.drain()

    nc.compile()
    return nc


# ======================= host-side sharding =======================

def shard_inputs(x, w_qkv, b_qkv, w_out, logit_scale):
    x = np.ascontiguousarray(np.asarray(x, dtype=np.float32))
    w_qkv = np.asarray(w_qkv, dtype=np.float32)
    b_qkv = np.asarray(b_qkv, dtype=np.float32)
    w_out = np.asarray(w_out, dtype=np.float32)
    ls = np.asarray(logit_scale, dtype=np.float32).reshape(-1)
    s_all = np.exp(np.minimum(ls, LOG100)).astype(np.float32)

    Wq = w_qkv[0 * INNER:1 * INNER]
    Wk = w_qkv[1 * INNER:2 * INNER]
    Wv = w_qkv[2 * INNER:3 * INNER]
    bq = b_qkv[0 * INNER:1 * INNER]
    bk = b_qkv[1 * INNER:2 * INNER]
    bv = b_qkv[2 * INNER:3 * INNER]

    xT = [np.ascontiguousarray(x[b].T.astype(ml_bf16)) for b in range(B)]

    per_hg = {}
    merge = [True] * 4
    for hg in range(2):
        heads = list(range(hg * 8, hg * 8 + 8))
        rows, brows = [], []
        lnsb = np.zeros((4, 34, 1), dtype=np.float32)
        for p in range(4):
            g0, g1 = heads[2 * p], heads[2 * p + 1]
            if s_all[g0] != s_all[g1]:
                merge[p] = False
            rows += [Wq[g0 * 64:(g0 + 1) * 64], Wq[g1 * 64:(g1 + 1) * 64],
                     Wk[g0 * 64:(g0 + 1) * 64], Wk[g1 * 64:(g1 + 1) * 64]]
            brows += [bq[g0 * 64:(g0 + 1) * 64], bq[g1 * 64:(g1 + 1) * 64],
                      bk[g0 * 64:(g0 + 1) * 64], bk[g1 * 64:(g1 + 1) * 64]]
            lnsb[p, 32, 0] = np.log(s_all[g0])
            lnsb[p, 33, 0] = np.log(s_all[g1])
        wqkT = np.ascontiguousarray(
            np.concatenate(rows, axis=0).T.astype(ml_bf16))
        bqk = np.ascontiguousarray(
            np.concatenate(brows, axis=0)).reshape(8, 128, 1)
        vsl = slice(hg * 512, (hg + 1) * 512)
        wvT = np.ascontiguousarray(Wv[vsl].T.astype(ml_bf16))
        bvs = np.ascontiguousarray(bv[vsl].reshape(1, 512))
        woT = np.ascontiguousarray(w_out[:, vsl].T.astype(ml_bf16))
        scl = np.stack([-s_all[heads], np.log(s_all[heads])],
                       axis=1).astype(np.float32).reshape(8, 2, 1)
        per_hg[hg] = dict(wqkT=wqkT, bqk=bqk, wvT=wvT, bv=bvs, woT=woT,
                          scl=scl, lnsb=lnsb)

    sel2 = np.zeros((2, 128), dtype=np.float32)
    sel2[0, 0:64] = 1.0
    sel2[1, 64:128] = 1.0
    sel2T = np.ascontiguousarray(sel2.T.astype(ml_bf16))
    in_maps = []
    for c in range(N_CORES):
        b, hg = c // 2, c % 2
        m = dict(per_hg[hg])
        m["xT"] = xT[b]
        m["sel2T"] = sel2T
        in_maps.append(m)
    return in_maps, tuple(merge)


_NC_CACHE = {}
TRACE = False
LAST_RESULT = None


def kernel(x, w_qkv, b_qkv, w_out, logit_scale):
    global LAST_RESULT
    in_maps, merge_pairs = shard_inputs(x, w_qkv, b_qkv, w_out, logit_scale)
    cfg = Cfg(merge_pairs=merge_pairs)
    if merge_pairs not in _NC_CACHE:
        _NC_CACHE[merge_pairs] = build(cfg)
    nc = _NC_CACHE[merge_pairs]
    res = run_bass_kernel_spmd(nc, in_maps, core_ids=list(range(N_CORES)),
                               trace=TRACE)
    LAST_RESULT = res
    outs = [res.results[c]["out"] for c in range(N_CORES)]
    full = np.empty((B, N, D_MODEL), dtype=np.float32)
    for b in range(B):
        full[b] = (outs[2 * b] + outs[2 * b + 1]).T
    return full
